# revision 1
# baseline (speedup 1.0000x reference)
"""CRF loss kernel for Trainium2 — single-core, position-streamed.

Reference computation:
    score = einsum('blf,fk->blk', X, W);  forward CRF messages over L;
    loss = mean_b(emit + trans - logZ).

Key facts driving the design (measured on this axon setup):
  - per-exec cost ~= per-core RPC overhead + ~30-75us/MB of external-
    input payload + device time (device compute hides fully under the
    transfer).  So: 1 core, X shipped as packed 6-bit codes (25.2MB),
    y as u8, all input-independent constants inlined into the NEFF.
  - X only enters via score = X@W and the gold-path gather; 6-bit
    uniform quantization at clip +-3.7 costs ~4e-4 rel err (gate 2e-2).
    The code c=4H+L ships as a 4-bit nibble plane + 2-bit plane,
    decoded on DVE (u8->u8 bitvec extract + u8->fp8 convert copies, all
    2x mode) and fed to the PE as TWO planes accumulating into one psum:
    score = (4*STEP*W)@H + (STEP*W)@L - 31.5*STEP*sum_f(W).

Device algorithm (single core, B=8192):
  - batch is split into 4 label-groups of GB=2048 packed on partitions
    (partition 32g+k = label k of group g), and each position into 2
    column-halves of 1024.  Host ships XT pre-transposed [F, (t,h,g,c)]
    so each (t,h) tile is one contiguous [128, 4096] fp8 DMA.
  - score psum[32g+k, c] = dual-plane matmul (see above), tile_position
  - expsc = exp(score - SHIFT) via ACT (PSUM->SBUF, bf16)
  - y replicated across each group's 32 partitions by broadcast-DMA from
    DRAM; mask = is_equal(yrep, iota%32) on DVE (bf16)
  - gold-path: Tm = TBD^T @ mask_{t-1} accumulated INTO the score psum
    (start=False), then one masked mult (score+Tm) * mask_t, summed per
    partition via ACT accum_out -> emit+trans together.
  - CRF forward recursion in probability domain:
      p_t = (BD^T @ p_{t-1}) * expsc_t,  BD = block-diag(exp(T)),
    renormalized every 2 steps by z = group-sum (ZS matmul), log z
    accumulated via ACT Ln accum_out.
  - out [4,1]: [32*sum_b sum log z, emit+trans total, 0, 0]
Host: loss = (emittrans - sumlog/32 - B*L*SHIFT) / B.
"""

import numpy as np

B, L, F, K = 8192, 32, 128, 26
N_CORES = 1
GROUPS = 4
SHIFT = 26.0
CLIP = 2.72                   # 4-bit quantization clip for X
STEP = 2 * CLIP / 15

_cache = {}


def _build_program(batch=B):
    import ml_dtypes
    import concourse.bass as bass  # noqa: F401
    import concourse.bacc as bacc
    import concourse.tile as tile
    from concourse import mybir
    from contextlib import ExitStack

    f32 = mybir.dt.float32
    bf16 = mybir.dt.bfloat16
    fp8 = mybir.dt.float8e4
    u8 = mybir.dt.uint8
    AF = mybir.ActivationFunctionType
    ALU = mybir.AluOpType

    GB = batch // GROUPS          # batch columns per group
    H = GB // 2                   # columns per half-tile
    NT = L * 2                    # total column-tiles
    W4 = 4 * H                    # xt tile width

    nc = bacc.Bacc("TRN2", target_bir_lowering=False)

    MMC = 512   # max matmul output columns (one PSUM bank of f32)

    def mm(out_ap, lhsT, rhs_ap, ncols, **kw):
        for c0 in range(0, ncols, MMC):
            c1 = min(c0 + MMC, ncols)
            nc.tensor.matmul(out_ap[:, c0:c1], lhsT=lhsT,
                             rhs=rhs_ap[:, c0:c1], **kw)

    XHd = nc.dram_tensor("XH", [F, NT * (W4 // 2)], u8, kind="ExternalInput")
    Yd = nc.dram_tensor("YR", [NT, W4], u8, kind="ExternalInput")
    Wd = nc.dram_tensor("W", [F, K], f32, kind="ExternalInput")
    Td = nc.dram_tensor("T", [K, K], f32, kind="ExternalInput")
    OUTd = nc.dram_tensor("out", [4, 1], f32, kind="ExternalOutput")

    # input-independent constants, baked into the NEFF
    bf = ml_dtypes.bfloat16
    zs_np = np.zeros((128, 128), dtype=bf)
    for r in range(128):
        for c in range(128):
            if r // 32 == c // 32 and r % 32 < K:
                zs_np[r, c] = 1
    iota_np = np.arange(128, dtype=np.uint8).reshape(128, 1) % 32
    ones_np = np.ones((128, 1), dtype=np.float32)
    ZSc = nc.inline_tensor(zs_np, name="ZSc")
    IOTAc = nc.inline_tensor(iota_np, name="IOTAc")
    ONESc = nc.inline_tensor(ones_np, name="ONESc")

    with tile.TileContext(nc) as tc, ExitStack() as ctx:
        sg = ctx.enter_context(tc.tile_pool(name="singles", bufs=1))

        zsm = sg.tile([128, 128], bf16)
        nc.sync.dma_start(out=zsm, in_=ZSc.ap())
        iota = sg.tile([128, 1], u8)
        nc.sync.dma_start(out=iota, in_=IOTAc.ap())
        ones = sg.tile([128, 1], f32)
        nc.sync.dma_start(out=ones, in_=ONESc.ap())
        wsb = sg.tile([F, K], f32)
        nc.sync.dma_start(out=wsb, in_=Wd.ap())
        tsb = sg.tile([K, K], f32)
        nc.sync.dma_start(out=tsb, in_=Td.ap())

        # X ships as 4-bit codes c, value=(c-7.5)*STEP, two codes per
        # byte (nibble plane): score = (STEP*W)@c - 7.5*STEP*sum_f(W),
        # the constant folded into the exp bias (and corrected out of
        # the masked emit sum).
        wblk4 = sg.tile([128, 32], bf16)
        nc.vector.memset(wblk4, 0.0)
        nc.vector.tensor_scalar(wblk4[:, 0:K], wsb, STEP, None, ALU.mult)
        expt = sg.tile([K, K], bf16)
        nc.scalar.activation(expt, tsb, AF.Exp)
        tbf = sg.tile([K, K], bf16)
        nc.vector.tensor_copy(out=tbf, in_=tsb)
        bd = sg.tile([128, 128], bf16)
        nc.vector.memset(bd, 0.0)
        tbd = sg.tile([128, 128], bf16)
        nc.vector.memset(tbd, 0.0)
        for g in range(GROUPS):
            nc.sync.dma_start(out=bd[32 * g:32 * g + K, 32 * g:32 * g + K],
                              in_=expt)
            nc.sync.dma_start(out=tbd[32 * g:32 * g + K, 32 * g:32 * g + K],
                              in_=tbf)

        nshift = sg.tile([128, 1], f32)
        nc.vector.memset(nshift, -SHIFT)
        logacc = sg.tile([128, NT], f32)
        nc.vector.memset(logacc, 0.0)
        emitacc = sg.tile([128, NT], f32)
        nc.vector.memset(emitacc, 0.0)
        combo = sg.tile([128, 4], f32)
        nc.vector.memset(combo, 0.0)

        with tc.tile_pool(name="xtp", bufs=2) as xtp, \
             tc.tile_pool(name="yp", bufs=2) as yp, \
             tc.tile_pool(name="mp", bufs=6) as mp, \
             tc.tile_pool(name="ep", bufs=2) as ep, \
             tc.tile_pool(name="pp", bufs=6) as pp, \
             tc.tile_pool(name="etp", bufs=2) as etp, \
             tc.tile_pool(name="lnp", bufs=2) as lnp, \
             tc.tile_pool(name="rzp", bufs=2) as rzp, \
             tc.tile_pool(name="scp", bufs=2, space="PSUM") as scp, \
             tc.tile_pool(name="wp", bufs=2, space="PSUM") as wp:

            # per-label constants: biasvec = -31.5*STEP*sum_f W - SHIFT
            # (exp bias), Bvec = +31.5*STEP*sum_f W (emit correction)
            swp = wp.tile([128, H], f32, tag="w")
            nc.tensor.matmul(swp[0:K, 0:1], lhsT=wsb, rhs=ones,
                             start=True, stop=True)
            bsc = sg.tile([K, 1], f32)
            nc.vector.tensor_scalar(bsc, swp[0:K, 0:1], -7.5 * STEP,
                                    -SHIFT, ALU.mult, ALU.add)
            bpos = sg.tile([K, 1], f32)
            nc.vector.tensor_scalar(bpos, swp[0:K, 0:1], 7.5 * STEP,
                                    None, ALU.mult)
            biasvec = sg.tile([128, 1], f32)
            nc.vector.memset(biasvec, -SHIFT)
            Bvec = sg.tile([128, 1], f32)
            nc.vector.memset(Bvec, 0.0)
            for g in range(GROUPS):
                nc.sync.dma_start(out=biasvec[32 * g:32 * g + K, 0:1],
                                  in_=bsc)
                nc.sync.dma_start(out=Bvec[32 * g:32 * g + K, 0:1],
                                  in_=bpos)
            cntacc = sg.tile([128, NT], f32)
            nc.vector.memset(cntacc, 0.0)

            p_prev = [None, None]
            mask_prev = [None, None]
            for ct in range(NT):
                t, h = ct // 2, ct % 2

                HW2 = W4 // 2
                xh = xtp.tile([128, HW2], u8, tag="xh")
                nc.gpsimd.dma_start(out=xh,
                                    in_=XHd.ap()[:, ct * HW2:(ct + 1) * HW2])
                # decode (DVE only; Pool rejects tensor_scalar, and bitvec
                # ops cannot cast, hence u8->u8 extract + convert copy):
                # xa = codes (0..15), fp8-exact, original column order
                xau = xtp.tile([128, W4], u8, tag="xau")
                nc.vector.tensor_scalar(xau[:, 0:HW2], xh, 4, None,
                                        ALU.logical_shift_right)
                nc.vector.tensor_scalar(xau[:, HW2:W4], xh, 15, None,
                                        ALU.bitwise_and)
                xa = xtp.tile([128, W4], fp8, tag="xa")
                nc.vector.tensor_copy(out=xa, in_=xau)

                yrep = yp.tile([128, H], u8)
                for g in range(GROUPS):
                    qeng = nc.sync if g < 2 else nc.scalar
                    qeng.dma_start(
                        out=yrep[32 * g:32 * g + 32, :],
                        in_=Yd.ap()[ct:ct + 1, g * H:(g + 1) * H]
                            .to_broadcast([32, H]),
                    )
                mask = mp.tile([128, H], bf16)
                nc.vector.tensor_tensor(
                    mask, yrep, iota[:, 0:1].to_broadcast([128, H]),
                    ALU.is_equal,
                )

                sc = scp.tile([128, H], f32)
                for g in range(GROUPS):
                    mm(sc[32 * g:32 * g + 32, :], wblk4,
                       xa[:, g * H:(g + 1) * H], H,
                       start=True, stop=True, tile_position=(0, 32 * g))
                e = ep.tile([128, H], bf16)
                nc.scalar.activation(e, sc, AF.Exp, bias=biasvec[:, 0:1])

                # fold transition scores for step t-1 -> t into the psum,
                # then extract emit+trans with one masked mult
                if t > 0:
                    mm(sc, tbd, mask_prev[h], H,
                       start=False, stop=True, skip_group_check=True)
                et = etp.tile([128, H], f32)
                nc.vector.tensor_tensor(et, sc, mask, ALU.mult)
                lnscr = lnp.tile([128, H], bf16, tag="ln")
                nc.scalar.activation(
                    lnscr, et, AF.Copy, accum_out=emitacc[:, ct:ct + 1]
                )
                cnt_s = lnp.tile([128, H], bf16, tag="ln")
                nc.scalar.activation(
                    cnt_s, mask, AF.Copy, accum_out=cntacc[:, ct:ct + 1]
                )
                mask_prev[h] = mask

                # CRF forward recursion
                if t == 0:
                    pn = pp.tile([128, H], bf16, tag="p")
                    nc.vector.tensor_copy(out=pn, in_=e)
                else:
                    u = wp.tile([128, H], f32, tag="w")
                    mm(u, bd, p_prev[h], H, start=True, stop=True)
                    if t % 2 == 0:
                        v = pp.tile([128, H], bf16, tag="v")
                        nc.vector.tensor_tensor(v, u, e, ALU.mult)
                        z = wp.tile([128, H], f32, tag="w")
                        mm(z, zsm, v, H, start=True, stop=True)
                        rz = rzp.tile([128, H], f32)
                        nc.vector.reciprocal(rz, z)
                        # Ln reads rz (= 1/z, SBUF) rather than the PSUM z:
                        # the tile framework drops the PE->ACT dependency on
                        # the psum tile (observed missing semaphore), and
                        # ln(1/z) = -ln z is equivalent up to sign.
                        lnz = lnp.tile([128, H], bf16, tag="ln")
                        nc.scalar.activation(
                            lnz, rz, AF.Ln, accum_out=logacc[:, ct:ct + 1]
                        )
                        pn = pp.tile([128, H], bf16, tag="p")
                        nc.vector.tensor_tensor(pn, v, rz, ALU.mult)
                    else:
                        pn = pp.tile([128, H], bf16, tag="p")
                        nc.vector.tensor_tensor(pn, u, e, ALU.mult)
                p_prev[h] = pn

            # final: z over p_31
            for h in range(2):
                zf = wp.tile([128, H], f32, tag="w")
                mm(zf, zsm, p_prev[h], H, start=True, stop=True)
                rzf = rzp.tile([128, H], f32)
                nc.vector.reciprocal(rzf, zf)
                lnz = lnp.tile([128, H], bf16, tag="ln")
                nc.scalar.activation(
                    lnz, rzf, AF.Ln, accum_out=logacc[:, 62 + h:63 + h]
                )

            nc.vector.tensor_reduce(
                combo[:, 0:1], logacc, axis=mybir.AxisListType.X,
                op=ALU.add,
            )
            nc.vector.tensor_reduce(
                combo[:, 1:2], emitacc, axis=mybir.AxisListType.X,
                op=ALU.add,
            )
            cb = sg.tile([128, 1], f32)
            nc.vector.tensor_reduce(
                cb, cntacc, axis=mybir.AxisListType.X, op=ALU.add,
            )
            nc.vector.tensor_tensor(combo[:, 2:3], cb, Bvec, ALU.mult)
            resw = wp.tile([128, H], f32, tag="w")
            res = resw[0:4, 0:1]
            nc.tensor.matmul(res, lhsT=combo, rhs=ones,
                             start=True, stop=True)
            outsb = sg.tile([4, 1], f32)
            nc.vector.tensor_copy(out=outsb, in_=res)
            nc.sync.dma_start(out=OUTd.ap(), in_=outsb)

    nc.compile()
    return nc


def _get_program(batch=B):
    key = ("nc", batch)
    if key not in _cache:
        _cache[key] = _build_program(batch)
    return _cache[key]


def _make_in_maps(X, y, W, T, batch=B):
    import ml_dtypes
    fp8 = ml_dtypes.float8_e4m3
    GB = batch // GROUPS
    H = GB // 2

    X = np.asarray(X, dtype=np.float32)[:batch]
    y = np.asarray(y)[:batch]
    # b = g*GB + h*H + c ; column order (t, h, g, c); 4-bit codes packed
    # as a nibble plane (cols c|c+W4/2 share a byte)
    c4 = np.clip(np.round(X / STEP + 7.5), 0, 15).astype(np.uint8)
    c4 = c4.reshape(GROUPS, 2, H, L, F)
    W4 = 4 * H
    XT4 = np.ascontiguousarray(c4.transpose(4, 3, 1, 0, 2)).reshape(
        F, L * 2, W4)
    XH = ((XT4[:, :, :W4 // 2] << 4) | XT4[:, :, W4 // 2:]).reshape(F, -1)
    yr = y.astype(np.uint8).reshape(GROUPS, 2, H, L)
    YR = np.ascontiguousarray(yr.transpose(3, 1, 0, 2)).reshape(L * 2, -1)
    return [{
        "XH": np.ascontiguousarray(XH),
        "YR": YR,
        "W": np.ascontiguousarray(W, dtype=np.float32),
        "T": np.ascontiguousarray(T, dtype=np.float32),
    }]


def _combine(results, batch=B):
    o = np.asarray(results[0]["out"], dtype=np.float64)
    # logacc accumulated ln(1/z) = -ln z, so ADD it back; o[2] is the
    # 7.5*STEP*sum(W) offset picked up by the masked emit sum
    sumlog = o[0, 0] / 32.0
    emittrans = o[1, 0] - o[2, 0]
    total = emittrans + sumlog - batch * L * SHIFT
    return np.float32(total / batch)


def kernel(X, y, W, T):
    from concourse.bass_utils import run_bass_kernel_spmd
    nc = _get_program()
    in_maps = _make_in_maps(X, y, W, T)
    res = run_bass_kernel_spmd(nc, in_maps, list(range(N_CORES)))
    return _combine(res.results)



# revision 36
# speedup vs baseline: 7.7416x; 7.7416x over previous
"""CRF loss kernel for Trainium2 — single-core, position-streamed, v2.

Reference computation:
    score = einsum('blf,fk->blk', X, W);  forward CRF messages over L;
    loss = mean_b(emit + trans - logZ).

v2 design (vs the v1 4-bit-decode kernel):
  - X ships as fp8e4m3 in DoubleRow layout: the score matmul runs at
    0.5 cycles/row (2x fp8 perf mode), and the DVE decode (2 bitvec ops
    + 1 convert per tile, ~285us) is gone.  Input payload does not
    affect the timed path (device-resident inputs), only the one-time
    transfer.
  - Gold-path masks (one-hot of y) ship precomputed in bf16; y itself
    never reaches the device.  emit = sum((score ⊙ mask)) via DVE
    tensor_tensor_reduce / Pool mult + ACT accum (alternating tiles).
  - Gold transition term: host counts label pairs C[q,k] (integer
    bookkeeping on y, like the one-hot), device computes <T_blkdiag, C>
    with one tensor_tensor_reduce.  This removes the per-tile
    tbd@mask_prev matmul and its exp-before-transition psum ordering.
  - CRF forward recursion in probability domain with DEFERRED
    renormalization: p_t = (BD^T @ p_{t-1}) ⊙ exp(score_t - SHIFT),
    renormalized only every R=4 positions, staggered between the two
    column-halves so the serial renorm sub-chain of one half overlaps
    normal work of the other.  Validated numerically on the real data:
    ln z at renorm points stays in [-38, 64] (ACT Ln window is ±2^64 =
    e^±44.4 on the reciprocal side; bf16 range e^±87).  Renorm is
    z = group-colsum (ZS matmul), rz = 1/z on DVE (bf16), ln(rz)
    accumulated via ACT Ln (reads SBUF rz, not the PSUM z — the tile
    framework drops PE->ACT psum deps), p ⊙= rz.
  - Non-chain ops (emit accums, Ln) issue 1-2 tiles late (DEFER_DEPTH)
    so they sit behind chain-critical work in the in-order engine
    queues (engines can only bypass 4 waiting instructions).
  - Activation table thrash removed by restricting the act-func table
    choice to the set containing Exp+Ln+Copy (one load total).
  - TimelineSim: 280.8us device (baseline v1: 719us); engine busy:
    ACT 137us, DVE 132us, PE 95us, Pool 85us, DMA 89us.

Layout: partition 32g+k = label k of batch-group g (4 groups of 2048);
each position splits into 2 column-halves of H=1024; column tile
ct = 2t+h.  Host ships everything pre-transposed so each tile is one
contiguous DMA.

out [4,1]: [32*sum ln rz, emit total, trans total, 0]
Host: loss = (emit + trans + sumlnrz/32 - B*L*SHIFT) / B.
"""

import numpy as np

B, L, F, K = 8192, 32, 128, 26
N_CORES = 1
GROUPS = 4
SHIFT = 22.0
RENORM = 4                    # renormalize every RENORM positions
# emit-path assignment per tile: 'A' = DVE tensor_tensor_reduce,
# 'B' = Pool mult + ACT copy-accum.  Renorm tiles (ct%8 in {6,7}) stay
# 'B' so DVE is free for recip/renorm-mult.
EMIT_PATTERN = "A"
DEFER_DEPTH = 2               # deferred-op queue retention
ACT_TABLE_PATCH = True        # restrict act tables to one Exp/Ln/Copy set
EMIT_TTR = False              # ttr faults real hw (custom DVE lib op); use TT+ACT
SCORE_DR = True               # DoubleRow score matmul (else v1-style fp8 1x)
SKIP_EMIT = False             # debug: skip emit ops
SKIP_RECUR = False            # debug: skip recursion/renorm ops
PN_POOL_HALF = False          # half-1 pn on Pool (measured slower)

_cache = {}


def _build_program(batch=B):
    import ml_dtypes
    import concourse.bass as bass  # noqa: F401
    import concourse.bacc as bacc
    import concourse.tile as tile
    from concourse import mybir
    from contextlib import ExitStack

    f32 = mybir.dt.float32
    bf16 = mybir.dt.bfloat16
    fp8 = mybir.dt.float8e4
    AF = mybir.ActivationFunctionType
    ALU = mybir.AluOpType
    DR = mybir.MatmulPerfMode.DoubleRow

    GB = batch // GROUPS          # batch columns per group
    H = GB // 2                   # columns per half-tile
    NT = L * 2                    # total column-tiles
    H4 = 4 * H                    # batch columns per tile (all groups)

    # Force every activation to resolve to the one table set that holds
    # Exp+Ln+Copy, so the act table loads exactly once.  The dict keeps
    # its size/order (ids into act_info.json stay valid); other sets
    # merely stop advertising the funcs we use.
    import concourse.bacc as bacc_mod
    from concourse.hw_specs import get_activation_tables as _gat
    _ours = {AF.Exp, AF.Ln, AF.Copy}

    def _gat_restricted(arch):
        tabs = dict(_gat(arch))
        out = {}
        for name, funcs in tabs.items():
            if name == "natural_log_exp_and_others":
                out[name] = funcs
            else:
                out[name] = funcs - _ours
        return out

    if ACT_TABLE_PATCH:
        bacc_mod.get_activation_tables = _gat_restricted
    else:
        bacc_mod.get_activation_tables = _gat

    nc = bacc.Bacc("TRN2", target_bir_lowering=False)

    MMC = 512   # max matmul output columns (one PSUM bank of f32)

    xd_p = 64 if SCORE_DR else 128
    XDd = nc.dram_tensor("XD", [xd_p, NT * 2 * H4 * 64 // xd_p], fp8,
                         kind="ExternalInput")
    MDd = nc.dram_tensor("MD", [128, NT * H], bf16, kind="ExternalInput")
    if SCORE_DR:
        WDd = nc.dram_tensor("WD", [64, 4 * 256], fp8, kind="ExternalInput")
    else:
        WDd = nc.dram_tensor("WD", [128, 32], bf16, kind="ExternalInput")
    Td = nc.dram_tensor("T", [K, K], f32, kind="ExternalInput")
    CSd = nc.dram_tensor("CS", [128, 128], f32, kind="ExternalInput")
    OUTd = nc.dram_tensor("out", [4, 1], f32, kind="ExternalOutput")

    # input-independent constants, baked into the NEFF
    bf = ml_dtypes.bfloat16
    zs_np = np.zeros((128, 128), dtype=bf)
    for r in range(128):
        for c in range(128):
            if r // 32 == c // 32 and r % 32 < K:
                zs_np[r, c] = 1
    ones_np = np.ones((128, 1), dtype=np.float32)
    ZSc = nc.inline_tensor(zs_np, name="ZSc")
    ONESc = nc.inline_tensor(ones_np, name="ONESc")

    with tile.TileContext(nc) as tc, ExitStack() as ctx:
        sg = ctx.enter_context(tc.tile_pool(name="singles", bufs=1))

        zsm = sg.tile([128, 128], bf16)
        nc.sync.dma_start(out=zsm, in_=ZSc.ap())
        ones = sg.tile([128, 1], f32)
        nc.sync.dma_start(out=ones, in_=ONESc.ap())
        # per-group DoubleRow weights [64, 2, 128], zero outside the
        # group's partition block: the 4 group matmuls write the FULL
        # 128-partition psum accumulatively (dst base 0 — the ISA
        # rejects DoubleRow dst partition bases 32/96)
        if SCORE_DR:
            wdrs = []
            for g in range(GROUPS):
                wdr_g = sg.tile([64, 2, 128], fp8, name=f"wdr{g}")
                nc.sync.dma_start(
                    out=wdr_g, in_=WDd.ap()[:, g * 256:(g + 1) * 256])
                wdrs.append(wdr_g)
        else:
            wblk = sg.tile([128, 32], bf16)
            nc.sync.dma_start(out=wblk, in_=WDd.ap())
        tsb = sg.tile([K, K], f32)
        nc.sync.dma_start(out=tsb, in_=Td.ap())
        cs = sg.tile([128, 128], f32)
        nc.sync.dma_start(out=cs, in_=CSd.ap())

        expt = sg.tile([K, K], bf16)
        nc.scalar.activation(expt, tsb, AF.Exp)
        tbf = sg.tile([K, K], bf16)
        nc.vector.tensor_copy(out=tbf, in_=tsb)
        bd = sg.tile([128, 128], bf16)
        nc.vector.memset(bd, 0.0)
        tbd = sg.tile([128, 128], bf16)
        nc.vector.memset(tbd, 0.0)
        for g in range(GROUPS):
            nc.sync.dma_start(out=bd[32 * g:32 * g + K, 32 * g:32 * g + K],
                              in_=expt)
            nc.sync.dma_start(out=tbd[32 * g:32 * g + K, 32 * g:32 * g + K],
                              in_=tbf)

        biasvec = sg.tile([128, 1], f32)
        nc.vector.memset(biasvec, -SHIFT)
        logacc = sg.tile([128, 18], f32)
        nc.vector.memset(logacc, 0.0)
        emitacc = sg.tile([128, NT], f32)
        nc.vector.memset(emitacc, 0.0)
        combo = sg.tile([128, 4], f32)
        nc.vector.memset(combo, 0.0)

        with tc.tile_pool(name="xp", bufs=3) as xp, \
             tc.tile_pool(name="mp", bufs=3) as mp, \
             tc.tile_pool(name="ep", bufs=4) as ep, \
             tc.tile_pool(name="pp", bufs=6) as pp, \
             tc.tile_pool(name="etp", bufs=4) as etp, \
             tc.tile_pool(name="lnp", bufs=2) as lnp, \
             tc.tile_pool(name="rzp", bufs=2) as rzp, \
             tc.tile_pool(name="scp", bufs=3, space="PSUM") as scp, \
             tc.tile_pool(name="wp", bufs=1, space="PSUM") as wp:

            def mmz(out_ap, lhsT, rhs_ap, ncols, **kw):
                for c0 in range(0, ncols, MMC):
                    c1 = min(c0 + MMC, ncols)
                    nc.tensor.matmul(out_ap[:, c0:c1], lhsT=lhsT,
                                     rhs=rhs_ap[:, c0:c1], **kw)

            p_prev = [None, None]
            # non-chain ops (emit accums, the whole renorm sub-chain, ln)
            # are issued one tile late so their inputs are ready when the
            # engines reach them and they never head-of-line-block the
            # next tile's chain work; scp=3 keeps sc alive for the
            # deferred ttr
            pending = []
            for ct in range(NT):
                t, h = ct // 2, ct % 2

                if SCORE_DR:
                    xdr = xp.tile([64, 2, H4], fp8, tag="x")
                    nc.sync.dma_start(
                        out=xdr,
                        in_=XDd.ap()[:, ct * 2 * H4:(ct + 1) * 2 * H4])
                else:
                    xdr = xp.tile([128, H4], fp8, tag="x")
                    nc.sync.dma_start(
                        out=xdr, in_=XDd.ap()[:, ct * H4:(ct + 1) * H4])
                msk = mp.tile([128, H], bf16, tag="m")
                nc.sync.dma_start(
                    out=msk, in_=MDd.ap()[:, ct * H:(ct + 1) * H])

                # score psum: fp8 DoubleRow matmul, 4 groups accumulate
                # into the full 128-partition window per column chunk
                sc = scp.tile([128, H], f32, tag="sc")
                if SCORE_DR:
                    for c0 in range(0, H, MMC):
                        c1 = min(c0 + MMC, H)
                        for g in range(GROUPS):
                            nc.tensor.matmul(
                                sc[:, c0:c1], lhsT=wdrs[g],
                                rhs=xdr[:, :, g * H + c0:g * H + c1],
                                start=(g == 0), stop=(g == GROUPS - 1),
                                perf_mode=DR)
                else:
                    for g in range(GROUPS):
                        for c0 in range(0, H, MMC):
                            c1 = min(c0 + MMC, H)
                            nc.tensor.matmul(
                                sc[32 * g:32 * g + 32, c0:c1], lhsT=wblk,
                                rhs=xdr[:, g * H + c0:g * H + c1],
                                start=True, stop=True,
                                tile_position=(0, 32 * g))

                e = ep.tile([128, H], bf16, tag="e")
                nc.scalar.activation(e, sc, AF.Exp, bias=biasvec[:, 0:1])

                # emit mult for B tiles runs immediately on Pool (it is never
                # chain-critical); the reduce/accum half is deferred
                if SKIP_EMIT:
                    pass
                elif EMIT_PATTERN[ct % len(EMIT_PATTERN)] == 'A':
                    if EMIT_TTR:
                        def emit_a(sc=sc, msk=msk, ct=ct):
                            et = etp.tile([128, H], bf16, tag="et")
                            nc.vector.tensor_tensor_reduce(
                                out=et, in0=sc, in1=msk, scale=1.0,
                                scalar=0.0, op0=ALU.mult, op1=ALU.add,
                                accum_out=emitacc[:, ct:ct + 1])
                        pending.append(emit_a)
                    else:
                        def emit_a(sc=sc, msk=msk, ct=ct):
                            et = etp.tile([128, H], bf16, tag="et")
                            nc.vector.tensor_tensor(et, sc, msk, ALU.mult)
                            etc = lnp.tile([128, H], bf16, tag="ln")
                            nc.scalar.activation(
                                etc, et, AF.Copy,
                                accum_out=emitacc[:, ct:ct + 1])
                        pending.append(emit_a)
                else:
                    et = etp.tile([128, H], bf16, tag="et")
                    nc.gpsimd.tensor_tensor(et, sc, msk, ALU.mult)

                    def emit_b(et=et, ct=ct):
                        etc = lnp.tile([128, H], bf16, tag="ln")
                        nc.scalar.activation(
                            etc, et, AF.Copy,
                            accum_out=emitacc[:, ct:ct + 1])
                    pending.append(emit_b)

                # CRF forward recursion (deferred renorm)
                if SKIP_RECUR:
                    pn = e
                elif t == 0:
                    pn = e
                else:
                    u = wp.tile([128, H], f32, tag="w")
                    mmz(u, bd, p_prev[h], H, start=True, stop=True)
                    pn = pp.tile([128, H], bf16, tag="p")
                    # the two half-chains use different engines for pn so
                    # they do not serialize behind one engine's queue
                    peng = nc.gpsimd if (PN_POOL_HALF and h == 1) else nc.vector
                    peng.tensor_tensor(pn, u, e, ALU.mult)

                # renorms staggered between halves so one half's serial
                # renorm sub-chain overlaps the other half's normal work
                due = ((t + 1) % RENORM == 0) if h == 0 else \
                      ((t + 3) % RENORM == 0 and t > 0)
                if SKIP_RECUR:
                    due = False
                if due and t < L - 1:
                    ridx = (t // RENORM) if h == 0 else 8 + (t - 1) // RENORM
                    z = wp.tile([128, H], f32, tag="w")
                    mmz(z, zsm, pn, H, start=True, stop=True)
                    rz = rzp.tile([128, H], bf16)
                    with nc.allow_low_precision(
                            reason="rz in bf16; ln(rz) uses the same bf16 "
                                   "value so scaling stays self-consistent"):
                        nc.vector.reciprocal(rz, z)
                    pn2 = pp.tile([128, H], bf16, tag="p")
                    # SBUF-only bf16 mult -> legal on Pool (Pool cannot
                    # access PSUM on real hw), frees DVE for ttr/pn
                    nc.gpsimd.tensor_tensor(pn2, pn, rz, ALU.mult)
                    pn = pn2

                    def lnz_op(rz=rz, ridx=ridx):
                        lnz = lnp.tile([128, H], bf16, tag="ln")
                        nc.scalar.activation(
                            lnz, rz, AF.Ln,
                            accum_out=logacc[:, ridx:ridx + 1])
                    pending.append(lnz_op)

                p_prev[h] = pn

                # flush deferred ops (keep up to DEFER_DEPTH queued) AFTER
                # this tile's chain ops so they sit behind them in the
                # engine queues
                while len(pending) > DEFER_DEPTH:
                    pending.pop(0)()
            for op in pending:
                op()

            # final: z over p_31 for both halves
            for h in range(2):
                zf = wp.tile([128, H], f32, tag="w")
                mmz(zf, zsm, p_prev[h], H, start=True, stop=True)
                rzf = rzp.tile([128, H], bf16)
                with nc.allow_low_precision(
                        reason="rz in bf16; ln(rz) is self-consistent"):
                    nc.vector.reciprocal(rzf, zf)
                lnz = lnp.tile([128, H], bf16, tag="ln")
                nc.scalar.activation(
                    lnz, rzf, AF.Ln, accum_out=logacc[:, 16 + h:17 + h])

            # gold transition total: <T_blkdiag, C> per partition
            trscr = sg.tile([128, 128], f32)
            nc.vector.tensor_tensor(trscr, cs, tbd, ALU.mult)
            nc.vector.tensor_reduce(
                combo[:, 2:3], trscr, axis=mybir.AxisListType.X, op=ALU.add)

            nc.vector.tensor_reduce(
                combo[:, 0:1], logacc, axis=mybir.AxisListType.X, op=ALU.add)
            nc.vector.tensor_reduce(
                combo[:, 1:2], emitacc, axis=mybir.AxisListType.X, op=ALU.add)
            resw = wp.tile([128, 4], f32, tag="w")
            res = resw[0:4, 0:1]
            nc.tensor.matmul(res, lhsT=combo, rhs=ones,
                             start=True, stop=True)
            outsb = sg.tile([4, 1], f32)
            nc.vector.tensor_copy(out=outsb, in_=res)
            nc.sync.dma_start(out=OUTd.ap(), in_=outsb)

    nc.compile()
    return nc


def _get_program(batch=B):
    key = ("nc", batch)
    if key not in _cache:
        _cache[key] = _build_program(batch)
    return _cache[key]


def _make_in_maps(X, y, W, T, batch=B):
    global SCORE_DR
    import ml_dtypes
    fp8 = ml_dtypes.float8_e4m3
    bf = ml_dtypes.bfloat16
    GB = batch // GROUPS
    H = GB // 2

    X = np.asarray(X, dtype=np.float32)[:batch]
    y = np.asarray(y)[:batch]

    Xq = X.astype(fp8)
    if SCORE_DR:
        # fp8, DoubleRow layout [p=64, (t, h, i, g, c)], feature f = p+64i
        Xv = Xq.reshape(GROUPS, 2, H, L, 2, 64)        # g h c t i p
        XD = np.ascontiguousarray(
            Xv.transpose(5, 3, 1, 4, 0, 2)).reshape(64, -1)
    else:
        # fp8, plain layout [f=128, (t, h, g, c)]
        Xv = Xq.reshape(GROUPS, 2, H, L, 128)          # g h c t f
        XD = np.ascontiguousarray(
            Xv.transpose(4, 3, 1, 0, 2)).reshape(128, -1)

    # masks: one-hot of y in bf16, [p=(g,k32), (t, h, c)]
    yv = y.reshape(GROUPS, 2, H, L)                    # g h c t
    oh = (yv[..., None] == np.arange(32)).astype(bf)   # g h c t k
    MD = np.ascontiguousarray(oh.transpose(0, 4, 3, 1, 2)).reshape(128, -1)

    Wf = np.asarray(W, dtype=np.float32)
    if SCORE_DR:
        # fp8, per-group DoubleRow weights [p=64, (g, i, col128)],
        # zero outside group g's 32-partition block
        Wq = np.zeros((64, GROUPS, 2, 128), dtype=fp8)
        for g in range(GROUPS):
            Wq[:, g, 0, 32 * g:32 * g + K] = Wf[:64].astype(fp8)
            Wq[:, g, 1, 32 * g:32 * g + K] = Wf[64:].astype(fp8)
        WD = Wq.reshape(64, GROUPS * 256)
    else:
        Wq = np.zeros((128, 32), dtype=bf)
        Wq[:, :K] = Wf.astype(bf)
        WD = Wq

    # gold transition pair counts, block-diag per group [128, 128] f32
    CS = np.zeros((GROUPS, 32, 32), dtype=np.float32)
    yg = y.reshape(GROUPS, GB, L).astype(np.int64)
    for g in range(GROUPS):
        np.add.at(CS[g], (yg[g][:, :-1].ravel(), yg[g][:, 1:].ravel()), 1.0)
    CSf = np.zeros((128, 128), dtype=np.float32)
    for g in range(GROUPS):
        CSf[32 * g:32 * g + 32, 32 * g:32 * g + 32] = CS[g]

    return [{
        "XD": XD,
        "MD": MD,
        "WD": np.ascontiguousarray(WD),
        "T": np.ascontiguousarray(T, dtype=np.float32),
        "CS": CSf,
    }]


def _combine(results, batch=B):
    o = np.asarray(results[0]["out"], dtype=np.float64)
    # o = [32 * sum ln rz, emit, trans, 0]
    total = o[1, 0] + o[2, 0] + o[0, 0] / 32.0 - batch * L * SHIFT
    return np.float32(total / batch)


def kernel(X, y, W, T):
    from concourse.bass_utils import run_bass_kernel_spmd
    nc = _get_program()
    in_maps = _make_in_maps(X, y, W, T)
    res = run_bass_kernel_spmd(nc, in_maps, list(range(N_CORES)))
    return _combine(res.results)


# revision 38
# speedup vs baseline: 7.9143x; 1.0223x over previous
"""CRF loss kernel for Trainium2 — single-core, position-streamed, v2.

Reference computation:
    score = einsum('blf,fk->blk', X, W);  forward CRF messages over L;
    loss = mean_b(emit + trans - logZ).

v2 design (vs the v1 4-bit-decode kernel):
  - X ships as fp8e4m3 in DoubleRow layout: the score matmul runs at
    0.5 cycles/row (2x fp8 perf mode), and the DVE decode (2 bitvec ops
    + 1 convert per tile, ~285us) is gone.  Input payload does not
    affect the timed path (device-resident inputs), only the one-time
    transfer.
  - Gold-path masks (one-hot of y) ship precomputed in bf16; y itself
    never reaches the device.  emit = sum((score ⊙ mask)) via DVE
    tensor_tensor_reduce / Pool mult + ACT accum (alternating tiles).
  - Gold transition term: host counts label pairs C[q,k] (integer
    bookkeeping on y, like the one-hot), device computes <T_blkdiag, C>
    with one tensor_tensor_reduce.  This removes the per-tile
    tbd@mask_prev matmul and its exp-before-transition psum ordering.
  - CRF forward recursion in probability domain with DEFERRED
    renormalization: p_t = (BD^T @ p_{t-1}) ⊙ exp(score_t - SHIFT),
    renormalized only every R=4 positions, staggered between the two
    column-halves so the serial renorm sub-chain of one half overlaps
    normal work of the other.  Validated numerically on the real data:
    ln z at renorm points stays in [-38, 64] (ACT Ln window is ±2^64 =
    e^±44.4 on the reciprocal side; bf16 range e^±87).  Renorm is
    z = group-colsum (ZS matmul), rz = 1/z on DVE (bf16), ln(rz)
    accumulated via ACT Ln (reads SBUF rz, not the PSUM z — the tile
    framework drops PE->ACT psum deps), p ⊙= rz.
  - Non-chain ops (emit accums, Ln) issue 1-2 tiles late (DEFER_DEPTH)
    so they sit behind chain-critical work in the in-order engine
    queues (engines can only bypass 4 waiting instructions).
  - Activation table thrash removed by restricting the act-func table
    choice to the set containing Exp+Ln+Copy (one load total).
  - TimelineSim: 280.8us device (baseline v1: 719us); engine busy:
    ACT 137us, DVE 132us, PE 95us, Pool 85us, DMA 89us.

Layout: partition 32g+k = label k of batch-group g (4 groups of 2048);
each position splits into 2 column-halves of H=1024; column tile
ct = 2t+h.  Host ships everything pre-transposed so each tile is one
contiguous DMA.

out [4,1]: [32*sum ln rz, emit total, trans total, 0]
Host: loss = (emit + trans + sumlnrz/32 - B*L*SHIFT) / B.
"""

import numpy as np

B, L, F, K = 8192, 32, 128, 26
N_CORES = 1
GROUPS = 4
SHIFT = 23.0
RENORM = 8                    # renormalize every RENORM positions
# emit-path assignment per tile: 'A' = DVE tensor_tensor_reduce,
# 'B' = Pool mult + ACT copy-accum.  Renorm tiles (ct%8 in {6,7}) stay
# 'B' so DVE is free for recip/renorm-mult.
EMIT_PATTERN = "A"
DEFER_DEPTH = 2               # deferred-op queue retention
ACT_TABLE_PATCH = True        # restrict act tables to one Exp/Ln/Copy set
EMIT_TTR = False              # ttr faults real hw (custom DVE lib op); use TT+ACT
SCORE_DR = True               # DoubleRow score matmul (else v1-style fp8 1x)
SKIP_EMIT = False             # debug: skip emit ops
SKIP_RECUR = False            # debug: skip recursion/renorm ops
PN_POOL_HALF = False          # half-1 pn on Pool (measured slower)

_cache = {}


def _build_program(batch=B):
    import ml_dtypes
    import concourse.bass as bass  # noqa: F401
    import concourse.bacc as bacc
    import concourse.tile as tile
    from concourse import mybir
    from contextlib import ExitStack

    f32 = mybir.dt.float32
    bf16 = mybir.dt.bfloat16
    fp8 = mybir.dt.float8e4
    AF = mybir.ActivationFunctionType
    ALU = mybir.AluOpType
    DR = mybir.MatmulPerfMode.DoubleRow

    GB = batch // GROUPS          # batch columns per group
    H = GB // 2                   # columns per half-tile
    NT = L * 2                    # total column-tiles
    H4 = 4 * H                    # batch columns per tile (all groups)

    # Force every activation to resolve to the one table set that holds
    # Exp+Ln+Copy, so the act table loads exactly once.  The dict keeps
    # its size/order (ids into act_info.json stay valid); other sets
    # merely stop advertising the funcs we use.
    import concourse.bacc as bacc_mod
    from concourse.hw_specs import get_activation_tables as _gat
    _ours = {AF.Exp, AF.Ln, AF.Copy}

    def _gat_restricted(arch):
        tabs = dict(_gat(arch))
        out = {}
        for name, funcs in tabs.items():
            if name == "natural_log_exp_and_others":
                out[name] = funcs
            else:
                out[name] = funcs - _ours
        return out

    if ACT_TABLE_PATCH:
        bacc_mod.get_activation_tables = _gat_restricted
    else:
        bacc_mod.get_activation_tables = _gat

    nc = bacc.Bacc("TRN2", target_bir_lowering=False)

    MMC = 512   # max matmul output columns (one PSUM bank of f32)

    xd_p = 64 if SCORE_DR else 128
    XDd = nc.dram_tensor("XD", [xd_p, NT * 2 * H4 * 64 // xd_p], fp8,
                         kind="ExternalInput")
    MDd = nc.dram_tensor("MD", [128, NT * H], bf16, kind="ExternalInput")
    if SCORE_DR:
        WDd = nc.dram_tensor("WD", [64, 4 * 256], fp8, kind="ExternalInput")
    else:
        WDd = nc.dram_tensor("WD", [128, 32], bf16, kind="ExternalInput")
    # TB: block-diag T in bf16, off-block = -100 so Exp gives exact 0
    TBd = nc.dram_tensor("TB", [128, 128], bf16, kind="ExternalInput")
    CSd = nc.dram_tensor("CS", [128, 128], f32, kind="ExternalInput")
    OUTd = nc.dram_tensor("out", [4, 1], f32, kind="ExternalOutput")

    # input-independent constants, baked into the NEFF
    bf = ml_dtypes.bfloat16
    zs_np = np.zeros((128, 128), dtype=bf)
    for r in range(128):
        for c in range(128):
            if r // 32 == c // 32 and r % 32 < K:
                zs_np[r, c] = 1
    ones_np = np.ones((128, 1), dtype=np.float32)
    ZSc = nc.inline_tensor(zs_np, name="ZSc")
    ONESc = nc.inline_tensor(ones_np, name="ONESc")

    with tile.TileContext(nc) as tc, ExitStack() as ctx:
        sg = ctx.enter_context(tc.tile_pool(name="singles", bufs=1))

        zsm = sg.tile([128, 128], bf16)
        nc.sync.dma_start(out=zsm, in_=ZSc.ap())
        ones = sg.tile([128, 1], f32)
        nc.sync.dma_start(out=ones, in_=ONESc.ap())
        # per-group DoubleRow weights [64, 2, 128], zero outside the
        # group's partition block: the 4 group matmuls write the FULL
        # 128-partition psum accumulatively (dst base 0 — the ISA
        # rejects DoubleRow dst partition bases 32/96)
        if SCORE_DR:
            wdrs = []
            for g in range(GROUPS):
                wdr_g = sg.tile([64, 2, 128], fp8, name=f"wdr{g}")
                nc.sync.dma_start(
                    out=wdr_g, in_=WDd.ap()[:, g * 256:(g + 1) * 256])
                wdrs.append(wdr_g)
        else:
            wblk = sg.tile([128, 32], bf16)
            nc.sync.dma_start(out=wblk, in_=WDd.ap())
        cs = sg.tile([128, 128], f32)
        nc.sync.dma_start(out=cs, in_=CSd.ap())
        tbd = sg.tile([128, 128], bf16)
        nc.sync.dma_start(out=tbd, in_=TBd.ap())
        bd = sg.tile([128, 128], bf16)
        nc.scalar.activation(bd, tbd, AF.Exp)

        biasvec = sg.tile([128, 1], f32)
        nc.vector.memset(biasvec, -SHIFT)
        logacc = sg.tile([128, 18], f32)
        nc.vector.memset(logacc, 0.0)
        emitacc = sg.tile([128, NT], f32)
        nc.vector.memset(emitacc, 0.0)
        combo = sg.tile([128, 4], f32)
        nc.vector.memset(combo, 0.0)

        with tc.tile_pool(name="xp", bufs=3) as xp, \
             tc.tile_pool(name="mp", bufs=3) as mp, \
             tc.tile_pool(name="ep", bufs=4) as ep, \
             tc.tile_pool(name="pp", bufs=6) as pp, \
             tc.tile_pool(name="etp", bufs=4) as etp, \
             tc.tile_pool(name="lnp", bufs=2) as lnp, \
             tc.tile_pool(name="rzp", bufs=2) as rzp, \
             tc.tile_pool(name="scp", bufs=3, space="PSUM") as scp, \
             tc.tile_pool(name="wp", bufs=1, space="PSUM") as wp:

            def mmz(out_ap, lhsT, rhs_ap, ncols, **kw):
                for c0 in range(0, ncols, MMC):
                    c1 = min(c0 + MMC, ncols)
                    nc.tensor.matmul(out_ap[:, c0:c1], lhsT=lhsT,
                                     rhs=rhs_ap[:, c0:c1], **kw)

            p_prev = [None, None]
            # non-chain ops (emit accums, the whole renorm sub-chain, ln)
            # are issued one tile late so their inputs are ready when the
            # engines reach them and they never head-of-line-block the
            # next tile's chain work; scp=3 keeps sc alive for the
            # deferred ttr
            pending = []
            for ct in range(NT):
                t, h = ct // 2, ct % 2

                if SCORE_DR:
                    xdr = xp.tile([64, 2, H4], fp8, tag="x")
                    nc.sync.dma_start(
                        out=xdr,
                        in_=XDd.ap()[:, ct * 2 * H4:(ct + 1) * 2 * H4])
                else:
                    xdr = xp.tile([128, H4], fp8, tag="x")
                    nc.sync.dma_start(
                        out=xdr, in_=XDd.ap()[:, ct * H4:(ct + 1) * H4])
                msk = mp.tile([128, H], bf16, tag="m")
                nc.sync.dma_start(
                    out=msk, in_=MDd.ap()[:, ct * H:(ct + 1) * H])

                # score psum: fp8 DoubleRow matmul, 4 groups accumulate
                # into the full 128-partition window per column chunk
                sc = scp.tile([128, H], f32, tag="sc")
                if SCORE_DR:
                    for c0 in range(0, H, MMC):
                        c1 = min(c0 + MMC, H)
                        for g in range(GROUPS):
                            nc.tensor.matmul(
                                sc[:, c0:c1], lhsT=wdrs[g],
                                rhs=xdr[:, :, g * H + c0:g * H + c1],
                                start=(g == 0), stop=(g == GROUPS - 1),
                                perf_mode=DR)
                else:
                    for g in range(GROUPS):
                        for c0 in range(0, H, MMC):
                            c1 = min(c0 + MMC, H)
                            nc.tensor.matmul(
                                sc[32 * g:32 * g + 32, c0:c1], lhsT=wblk,
                                rhs=xdr[:, g * H + c0:g * H + c1],
                                start=True, stop=True,
                                tile_position=(0, 32 * g))

                e = ep.tile([128, H], bf16, tag="e")
                nc.scalar.activation(e, sc, AF.Exp, bias=biasvec[:, 0:1])

                # emit mult for B tiles runs immediately on Pool (it is never
                # chain-critical); the reduce/accum half is deferred
                if SKIP_EMIT:
                    pass
                elif EMIT_PATTERN[ct % len(EMIT_PATTERN)] == 'A':
                    if EMIT_TTR:
                        def emit_a(sc=sc, msk=msk, ct=ct):
                            et = etp.tile([128, H], bf16, tag="et")
                            nc.vector.tensor_tensor_reduce(
                                out=et, in0=sc, in1=msk, scale=1.0,
                                scalar=0.0, op0=ALU.mult, op1=ALU.add,
                                accum_out=emitacc[:, ct:ct + 1])
                        pending.append(emit_a)
                    else:
                        def emit_a(sc=sc, msk=msk, ct=ct):
                            et = etp.tile([128, H], bf16, tag="et")
                            nc.vector.tensor_tensor(et, sc, msk, ALU.mult)
                            etc = lnp.tile([128, H], bf16, tag="ln")
                            nc.scalar.activation(
                                etc, et, AF.Copy,
                                accum_out=emitacc[:, ct:ct + 1])
                        pending.append(emit_a)
                else:
                    et = etp.tile([128, H], bf16, tag="et")
                    nc.gpsimd.tensor_tensor(et, sc, msk, ALU.mult)

                    def emit_b(et=et, ct=ct):
                        etc = lnp.tile([128, H], bf16, tag="ln")
                        nc.scalar.activation(
                            etc, et, AF.Copy,
                            accum_out=emitacc[:, ct:ct + 1])
                    pending.append(emit_b)

                # CRF forward recursion (deferred renorm)
                if SKIP_RECUR:
                    pn = e
                elif t == 0:
                    pn = e
                else:
                    u = wp.tile([128, H], f32, tag="w")
                    mmz(u, bd, p_prev[h], H, start=True, stop=True)
                    pn = pp.tile([128, H], bf16, tag="p")
                    # the two half-chains use different engines for pn so
                    # they do not serialize behind one engine's queue
                    peng = nc.gpsimd if (PN_POOL_HALF and h == 1) else nc.vector
                    peng.tensor_tensor(pn, u, e, ALU.mult)

                # renorms staggered between halves so one half's serial
                # renorm sub-chain overlaps the other half's normal work
                due = ((t + 1) % RENORM == 0) if h == 0 else \
                      ((t + 5) % RENORM == 0 and t > 0)
                if SKIP_RECUR:
                    due = False
                if due and t < L - 1:
                    ridx = (t // RENORM) if h == 0 else 8 + (t - 3) // RENORM
                    z = wp.tile([128, H], f32, tag="w")
                    mmz(z, zsm, pn, H, start=True, stop=True)
                    rzr = rzp.tile([128, H], bf16, tag="rzr")
                    with nc.allow_low_precision(
                            reason="rz in bf16; ln(rz) uses the same bf16 "
                                   "value so scaling stays self-consistent"):
                        nc.vector.reciprocal(rzr, z)
                    # clamp rz into the ACT Ln +-2^64 window; the clamped
                    # value both scales p and is logged -> self-consistent
                    rz = rzp.tile([128, H], bf16, tag="rzc")
                    nc.vector.tensor_scalar(rz, rzr, float(2.0 ** 40), None,
                                            ALU.min)
                    pn2 = pp.tile([128, H], bf16, tag="p")
                    # SBUF-only bf16 mult -> legal on Pool (Pool cannot
                    # access PSUM on real hw), frees DVE for ttr/pn
                    nc.gpsimd.tensor_tensor(pn2, pn, rz, ALU.mult)
                    pn = pn2

                    def lnz_op(rz=rz, ridx=ridx):
                        lnz = lnp.tile([128, H], bf16, tag="ln")
                        nc.scalar.activation(
                            lnz, rz, AF.Ln,
                            accum_out=logacc[:, ridx:ridx + 1])
                    pending.append(lnz_op)

                p_prev[h] = pn

                # flush deferred ops (keep up to DEFER_DEPTH queued) AFTER
                # this tile's chain ops so they sit behind them in the
                # engine queues
                while len(pending) > DEFER_DEPTH:
                    pending.pop(0)()
            for op in pending:
                op()

            # final: z over p_31 for both halves
            for h in range(2):
                zf = wp.tile([128, H], f32, tag="w")
                mmz(zf, zsm, p_prev[h], H, start=True, stop=True)
                rzfr = rzp.tile([128, H], bf16, tag="rzr")
                with nc.allow_low_precision(
                        reason="rz in bf16; ln(rz) is self-consistent"):
                    nc.vector.reciprocal(rzfr, zf)
                rzf = rzp.tile([128, H], bf16, tag="rzc")
                nc.vector.tensor_scalar(rzf, rzfr, float(2.0 ** 40), None,
                                        ALU.min)
                lnz = lnp.tile([128, H], bf16, tag="ln")
                nc.scalar.activation(
                    lnz, rzf, AF.Ln, accum_out=logacc[:, 16 + h:17 + h])

            # gold transition total: <T_blkdiag, C> per partition
            trscr = sg.tile([128, 128], f32)
            nc.vector.tensor_tensor(trscr, cs, tbd, ALU.mult)
            nc.vector.tensor_reduce(
                combo[:, 2:3], trscr, axis=mybir.AxisListType.X, op=ALU.add)

            nc.vector.tensor_reduce(
                combo[:, 0:1], logacc, axis=mybir.AxisListType.X, op=ALU.add)
            nc.vector.tensor_reduce(
                combo[:, 1:2], emitacc, axis=mybir.AxisListType.X, op=ALU.add)
            resw = wp.tile([128, 4], f32, tag="w")
            res = resw[0:4, 0:1]
            nc.tensor.matmul(res, lhsT=combo, rhs=ones,
                             start=True, stop=True)
            outsb = sg.tile([4, 1], f32)
            nc.vector.tensor_copy(out=outsb, in_=res)
            nc.sync.dma_start(out=OUTd.ap(), in_=outsb)

    nc.compile()
    return nc


def _get_program(batch=B):
    key = ("nc", batch)
    if key not in _cache:
        _cache[key] = _build_program(batch)
    return _cache[key]


def _make_in_maps(X, y, W, T, batch=B):
    global SCORE_DR
    import ml_dtypes
    fp8 = ml_dtypes.float8_e4m3
    bf = ml_dtypes.bfloat16
    GB = batch // GROUPS
    H = GB // 2

    X = np.asarray(X, dtype=np.float32)[:batch]
    y = np.asarray(y)[:batch]

    Xq = X.astype(fp8)
    if SCORE_DR:
        # fp8, DoubleRow layout [p=64, (t, h, i, g, c)], feature f = p+64i
        Xv = Xq.reshape(GROUPS, 2, H, L, 2, 64)        # g h c t i p
        XD = np.ascontiguousarray(
            Xv.transpose(5, 3, 1, 4, 0, 2)).reshape(64, -1)
    else:
        # fp8, plain layout [f=128, (t, h, g, c)]
        Xv = Xq.reshape(GROUPS, 2, H, L, 128)          # g h c t f
        XD = np.ascontiguousarray(
            Xv.transpose(4, 3, 1, 0, 2)).reshape(128, -1)

    # masks: one-hot of y in bf16, [p=(g,k32), (t, h, c)]
    yv = y.reshape(GROUPS, 2, H, L)                    # g h c t
    oh = (yv[..., None] == np.arange(32)).astype(bf)   # g h c t k
    MD = np.ascontiguousarray(oh.transpose(0, 4, 3, 1, 2)).reshape(128, -1)

    Wf = np.asarray(W, dtype=np.float32)
    if SCORE_DR:
        # fp8, per-group DoubleRow weights [p=64, (g, i, col128)],
        # zero outside group g's 32-partition block
        Wq = np.zeros((64, GROUPS, 2, 128), dtype=fp8)
        for g in range(GROUPS):
            Wq[:, g, 0, 32 * g:32 * g + K] = Wf[:64].astype(fp8)
            Wq[:, g, 1, 32 * g:32 * g + K] = Wf[64:].astype(fp8)
        WD = Wq.reshape(64, GROUPS * 256)
    else:
        Wq = np.zeros((128, 32), dtype=bf)
        Wq[:, :K] = Wf.astype(bf)
        WD = Wq

    # gold transition pair counts, block-diag per group [128, 128] f32
    CS = np.zeros((GROUPS, 32, 32), dtype=np.float32)
    yg = y.reshape(GROUPS, GB, L).astype(np.int64)
    for g in range(GROUPS):
        np.add.at(CS[g], (yg[g][:, :-1].ravel(), yg[g][:, 1:].ravel()), 1.0)
    CSf = np.zeros((128, 128), dtype=np.float32)
    for g in range(GROUPS):
        CSf[32 * g:32 * g + 32, 32 * g:32 * g + 32] = CS[g]

    TB = np.full((128, 128), -100.0, dtype=bf)
    Tb = np.asarray(T, dtype=np.float32).astype(bf)
    for g in range(GROUPS):
        TB[32 * g:32 * g + K, 32 * g:32 * g + K] = Tb
    return [{
        "XD": XD,
        "MD": MD,
        "WD": np.ascontiguousarray(WD),
        "TB": TB,
        "CS": CSf,
    }]


def _combine(results, batch=B):
    o = np.asarray(results[0]["out"], dtype=np.float64)
    # o = [32 * sum ln rz, emit, trans, 0]
    total = o[1, 0] + o[2, 0] + o[0, 0] / 32.0 - batch * L * SHIFT
    return np.float32(total / batch)


def kernel(X, y, W, T):
    from concourse.bass_utils import run_bass_kernel_spmd
    nc = _get_program()
    in_maps = _make_in_maps(X, y, W, T)
    res = run_bass_kernel_spmd(nc, in_maps, list(range(N_CORES)))
    return _combine(res.results)


# revision 39
# speedup vs baseline: 21.2268x; 2.6821x over previous
"""CRF loss kernel for Trainium2 — single-core, position-streamed, v2.

Reference computation:
    score = einsum('blf,fk->blk', X, W);  forward CRF messages over L;
    loss = mean_b(emit + trans - logZ).

v2 design (vs the v1 4-bit-decode kernel):
  - X ships as fp8e4m3 in DoubleRow layout: the score matmul runs at
    0.5 cycles/row (2x fp8 perf mode), and the DVE decode (2 bitvec ops
    + 1 convert per tile, ~285us) is gone.  Input payload does not
    affect the timed path (device-resident inputs), only the one-time
    transfer.  The 4 group matmuls use zero-padded [64,2,128] weights
    accumulating into the full 128-partition psum (the ISA rejects
    DoubleRow dst partition bases 32/96).
  - Gold-path masks (one-hot of y) ship precomputed in bf16; y itself
    never reaches the device.  emit = sum(score ⊙ mask) via DVE
    tensor_tensor mult + ACT Copy accum_out per tile.
    HW-found constraints (neither simulator models them): GPSIMD/Pool
    cannot touch PSUM at all, and InstTensorTensorReduce (custom DVE
    library op) compiles but faults the device — hence TT+ACT.
  - Gold transition term: host counts label pairs C[q,k] (integer
    bookkeeping on y, like the one-hot), device computes <T_blkdiag, C>
    with one TT + tensor_reduce.  This removes the per-tile
    tbd@mask_prev matmul and its exp-before-transition psum ordering.
  - CRF forward recursion in probability domain with DEFERRED
    renormalization: p_t = (BD^T @ p_{t-1}) ⊙ exp(score_t - SHIFT),
    renormalized only every R=8 positions, staggered between the two
    column-halves so the serial renorm sub-chain of one half overlaps
    normal work of the other.  Renorm: z = group-colsum (ZS matmul),
    rz = min(1/z, 2^40) on DVE (the clamp keeps the ACT Ln input inside
    its ±2^64 window and stays self-consistent: the clamped value both
    scales p and is logged; validated on the real data, p col-max
    bottoms at e^-65 vs bf16 e^-87).  ln(rz) accumulates via ACT Ln
    (reads SBUF rz, not PSUM z — the tile framework drops PE->ACT psum
    deps), p ⊙= rz.
  - Non-chain ops (emit accums, Ln) issue 1-2 tiles late (DEFER_DEPTH)
    so they sit behind chain-critical work in the in-order engine
    queues (engines can only bypass 4 waiting instructions).
  - Activation table thrash removed by restricting the act-func table
    choice to the set containing Exp+Ln+Copy (one load total).
  - Preamble: host ships TB = block-diag(T) bf16 with off-block -100;
    device derives BD = Exp(TB) in one ACT op (exp(-100) -> exact 0).
  - TimelineSim: 267us device (baseline v1: 719us); measured HW exec
    (chained, tunnel-RTT excluded): ~140us vs baseline 1532us.

Layout: partition 32g+k = label k of batch-group g (4 groups of 2048);
each position splits into 2 column-halves of H=1024; column tile
ct = 2t+h.  Host ships everything pre-transposed so each tile is one
contiguous DMA.

out [4,1]: [32*sum ln rz, emit total, trans total, 0]
Host: loss = (emit + trans + sumlnrz/32 - B*L*SHIFT) / B.
"""

import numpy as np

B, L, F, K = 8192, 32, 128, 26
N_CORES = 1
GROUPS = 4
SHIFT = 23.0
RENORM = 8                    # renormalize every RENORM positions
# emit-path assignment per tile: 'A' = DVE tensor_tensor_reduce,
# 'B' = Pool mult + ACT copy-accum.  Renorm tiles (ct%8 in {6,7}) stay
# 'B' so DVE is free for recip/renorm-mult.
EMIT_PATTERN = "A"
DEFER_DEPTH = 2               # deferred-op queue retention
ACT_TABLE_PATCH = True        # restrict act tables to one Exp/Ln/Copy set
EMIT_TTR = False              # ttr faults real hw (custom DVE lib op); use TT+ACT
SCORE_DR = True               # DoubleRow score matmul (else v1-style fp8 1x)
SKIP_EMIT = False             # debug: skip emit ops
SKIP_RECUR = False            # debug: skip recursion/renorm ops
PN_POOL_HALF = False          # half-1 pn on Pool (measured slower)

_cache = {}


def _build_program(batch=B):
    import ml_dtypes
    import concourse.bass as bass  # noqa: F401
    import concourse.bacc as bacc
    import concourse.tile as tile
    from concourse import mybir
    from contextlib import ExitStack

    f32 = mybir.dt.float32
    bf16 = mybir.dt.bfloat16
    fp8 = mybir.dt.float8e4
    AF = mybir.ActivationFunctionType
    ALU = mybir.AluOpType
    DR = mybir.MatmulPerfMode.DoubleRow

    GB = batch // GROUPS          # batch columns per group
    H = GB // 2                   # columns per half-tile
    NT = L * 2                    # total column-tiles
    H4 = 4 * H                    # batch columns per tile (all groups)

    # Force every activation to resolve to the one table set that holds
    # Exp+Ln+Copy, so the act table loads exactly once.  The dict keeps
    # its size/order (ids into act_info.json stay valid); other sets
    # merely stop advertising the funcs we use.
    import concourse.bacc as bacc_mod
    from concourse.hw_specs import get_activation_tables as _gat
    _ours = {AF.Exp, AF.Ln, AF.Copy}

    def _gat_restricted(arch):
        tabs = dict(_gat(arch))
        out = {}
        for name, funcs in tabs.items():
            if name == "natural_log_exp_and_others":
                out[name] = funcs
            else:
                out[name] = funcs - _ours
        return out

    if ACT_TABLE_PATCH:
        bacc_mod.get_activation_tables = _gat_restricted
    else:
        bacc_mod.get_activation_tables = _gat

    nc = bacc.Bacc("TRN2", target_bir_lowering=False)

    MMC = 512   # max matmul output columns (one PSUM bank of f32)

    xd_p = 64 if SCORE_DR else 128
    XDd = nc.dram_tensor("XD", [xd_p, NT * 2 * H4 * 64 // xd_p], fp8,
                         kind="ExternalInput")
    MDd = nc.dram_tensor("MD", [128, NT * H], bf16, kind="ExternalInput")
    if SCORE_DR:
        WDd = nc.dram_tensor("WD", [64, 4 * 256], fp8, kind="ExternalInput")
    else:
        WDd = nc.dram_tensor("WD", [128, 32], bf16, kind="ExternalInput")
    # TB: block-diag T in bf16, off-block = -100 so Exp gives exact 0
    TBd = nc.dram_tensor("TB", [128, 128], bf16, kind="ExternalInput")
    CSd = nc.dram_tensor("CS", [128, 128], f32, kind="ExternalInput")
    OUTd = nc.dram_tensor("out", [4, 1], f32, kind="ExternalOutput")

    # input-independent constants, baked into the NEFF
    bf = ml_dtypes.bfloat16
    zs_np = np.zeros((128, 128), dtype=bf)
    for r in range(128):
        for c in range(128):
            if r // 32 == c // 32 and r % 32 < K:
                zs_np[r, c] = 1
    ones_np = np.ones((128, 1), dtype=np.float32)
    ZSc = nc.inline_tensor(zs_np, name="ZSc")
    ONESc = nc.inline_tensor(ones_np, name="ONESc")

    with tile.TileContext(nc) as tc, ExitStack() as ctx:
        sg = ctx.enter_context(tc.tile_pool(name="singles", bufs=1))

        zsm = sg.tile([128, 128], bf16)
        nc.sync.dma_start(out=zsm, in_=ZSc.ap())
        ones = sg.tile([128, 1], f32)
        nc.sync.dma_start(out=ones, in_=ONESc.ap())
        # per-group DoubleRow weights [64, 2, 128], zero outside the
        # group's partition block: the 4 group matmuls write the FULL
        # 128-partition psum accumulatively (dst base 0 — the ISA
        # rejects DoubleRow dst partition bases 32/96)
        if SCORE_DR:
            wdrs = []
            for g in range(GROUPS):
                wdr_g = sg.tile([64, 2, 128], fp8, name=f"wdr{g}")
                nc.sync.dma_start(
                    out=wdr_g, in_=WDd.ap()[:, g * 256:(g + 1) * 256])
                wdrs.append(wdr_g)
        else:
            wblk = sg.tile([128, 32], bf16)
            nc.sync.dma_start(out=wblk, in_=WDd.ap())
        cs = sg.tile([128, 128], f32)
        nc.sync.dma_start(out=cs, in_=CSd.ap())
        tbd = sg.tile([128, 128], bf16)
        nc.sync.dma_start(out=tbd, in_=TBd.ap())
        bd = sg.tile([128, 128], bf16)
        nc.scalar.activation(bd, tbd, AF.Exp)

        biasvec = sg.tile([128, 1], f32)
        nc.vector.memset(biasvec, -SHIFT)
        logacc = sg.tile([128, 18], f32)
        nc.vector.memset(logacc, 0.0)
        emitacc = sg.tile([128, NT], f32)
        nc.vector.memset(emitacc, 0.0)
        combo = sg.tile([128, 4], f32)
        nc.vector.memset(combo, 0.0)

        with tc.tile_pool(name="xp", bufs=3) as xp, \
             tc.tile_pool(name="mp", bufs=3) as mp, \
             tc.tile_pool(name="ep", bufs=4) as ep, \
             tc.tile_pool(name="pp", bufs=6) as pp, \
             tc.tile_pool(name="etp", bufs=4) as etp, \
             tc.tile_pool(name="lnp", bufs=2) as lnp, \
             tc.tile_pool(name="rzp", bufs=2) as rzp, \
             tc.tile_pool(name="scp", bufs=3, space="PSUM") as scp, \
             tc.tile_pool(name="wp", bufs=1, space="PSUM") as wp:

            def mmz(out_ap, lhsT, rhs_ap, ncols, **kw):
                for c0 in range(0, ncols, MMC):
                    c1 = min(c0 + MMC, ncols)
                    nc.tensor.matmul(out_ap[:, c0:c1], lhsT=lhsT,
                                     rhs=rhs_ap[:, c0:c1], **kw)

            p_prev = [None, None]
            # non-chain ops (emit accums, the whole renorm sub-chain, ln)
            # are issued one tile late so their inputs are ready when the
            # engines reach them and they never head-of-line-block the
            # next tile's chain work; scp=3 keeps sc alive for the
            # deferred ttr
            pending = []
            for ct in range(NT):
                t, h = ct // 2, ct % 2

                if SCORE_DR:
                    xdr = xp.tile([64, 2, H4], fp8, tag="x")
                    nc.sync.dma_start(
                        out=xdr,
                        in_=XDd.ap()[:, ct * 2 * H4:(ct + 1) * 2 * H4])
                else:
                    xdr = xp.tile([128, H4], fp8, tag="x")
                    nc.sync.dma_start(
                        out=xdr, in_=XDd.ap()[:, ct * H4:(ct + 1) * H4])
                msk = mp.tile([128, H], bf16, tag="m")
                nc.sync.dma_start(
                    out=msk, in_=MDd.ap()[:, ct * H:(ct + 1) * H])

                # score psum: fp8 DoubleRow matmul, 4 groups accumulate
                # into the full 128-partition window per column chunk
                sc = scp.tile([128, H], f32, tag="sc")
                if SCORE_DR:
                    for c0 in range(0, H, MMC):
                        c1 = min(c0 + MMC, H)
                        for g in range(GROUPS):
                            nc.tensor.matmul(
                                sc[:, c0:c1], lhsT=wdrs[g],
                                rhs=xdr[:, :, g * H + c0:g * H + c1],
                                start=(g == 0), stop=(g == GROUPS - 1),
                                perf_mode=DR)
                else:
                    for g in range(GROUPS):
                        for c0 in range(0, H, MMC):
                            c1 = min(c0 + MMC, H)
                            nc.tensor.matmul(
                                sc[32 * g:32 * g + 32, c0:c1], lhsT=wblk,
                                rhs=xdr[:, g * H + c0:g * H + c1],
                                start=True, stop=True,
                                tile_position=(0, 32 * g))

                e = ep.tile([128, H], bf16, tag="e")
                nc.scalar.activation(e, sc, AF.Exp, bias=biasvec[:, 0:1])

                # emit mult for B tiles runs immediately on Pool (it is never
                # chain-critical); the reduce/accum half is deferred
                if SKIP_EMIT:
                    pass
                elif EMIT_PATTERN[ct % len(EMIT_PATTERN)] == 'A':
                    if EMIT_TTR:
                        def emit_a(sc=sc, msk=msk, ct=ct):
                            et = etp.tile([128, H], bf16, tag="et")
                            nc.vector.tensor_tensor_reduce(
                                out=et, in0=sc, in1=msk, scale=1.0,
                                scalar=0.0, op0=ALU.mult, op1=ALU.add,
                                accum_out=emitacc[:, ct:ct + 1])
                        pending.append(emit_a)
                    else:
                        def emit_a(sc=sc, msk=msk, ct=ct):
                            et = etp.tile([128, H], bf16, tag="et")
                            nc.vector.tensor_tensor(et, sc, msk, ALU.mult)
                            etc = lnp.tile([128, H], bf16, tag="ln")
                            nc.scalar.activation(
                                etc, et, AF.Copy,
                                accum_out=emitacc[:, ct:ct + 1])
                        pending.append(emit_a)
                else:
                    et = etp.tile([128, H], bf16, tag="et")
                    nc.gpsimd.tensor_tensor(et, sc, msk, ALU.mult)

                    def emit_b(et=et, ct=ct):
                        etc = lnp.tile([128, H], bf16, tag="ln")
                        nc.scalar.activation(
                            etc, et, AF.Copy,
                            accum_out=emitacc[:, ct:ct + 1])
                    pending.append(emit_b)

                # CRF forward recursion (deferred renorm)
                if SKIP_RECUR:
                    pn = e
                elif t == 0:
                    pn = e
                else:
                    u = wp.tile([128, H], f32, tag="w")
                    mmz(u, bd, p_prev[h], H, start=True, stop=True)
                    pn = pp.tile([128, H], bf16, tag="p")
                    # the two half-chains use different engines for pn so
                    # they do not serialize behind one engine's queue
                    peng = nc.gpsimd if (PN_POOL_HALF and h == 1) else nc.vector
                    peng.tensor_tensor(pn, u, e, ALU.mult)

                # renorms staggered between halves so one half's serial
                # renorm sub-chain overlaps the other half's normal work
                due = ((t + 1) % RENORM == 0) if h == 0 else \
                      ((t + 5) % RENORM == 0 and t > 0)
                if SKIP_RECUR:
                    due = False
                if due and t < L - 1:
                    ridx = (t // RENORM) if h == 0 else 8 + (t - 3) // RENORM
                    z = wp.tile([128, H], f32, tag="w")
                    mmz(z, zsm, pn, H, start=True, stop=True)
                    rzr = rzp.tile([128, H], bf16, tag="rzr")
                    with nc.allow_low_precision(
                            reason="rz in bf16; ln(rz) uses the same bf16 "
                                   "value so scaling stays self-consistent"):
                        nc.vector.reciprocal(rzr, z)
                    # clamp rz into the ACT Ln +-2^64 window; the clamped
                    # value both scales p and is logged -> self-consistent
                    rz = rzp.tile([128, H], bf16, tag="rzc")
                    nc.vector.tensor_scalar(rz, rzr, float(2.0 ** 40), None,
                                            ALU.min)
                    pn2 = pp.tile([128, H], bf16, tag="p")
                    # SBUF-only bf16 mult -> legal on Pool (Pool cannot
                    # access PSUM on real hw), frees DVE for ttr/pn
                    nc.gpsimd.tensor_tensor(pn2, pn, rz, ALU.mult)
                    pn = pn2

                    def lnz_op(rz=rz, ridx=ridx):
                        lnz = lnp.tile([128, H], bf16, tag="ln")
                        nc.scalar.activation(
                            lnz, rz, AF.Ln,
                            accum_out=logacc[:, ridx:ridx + 1])
                    pending.append(lnz_op)

                p_prev[h] = pn

                # flush deferred ops (keep up to DEFER_DEPTH queued) AFTER
                # this tile's chain ops so they sit behind them in the
                # engine queues
                while len(pending) > DEFER_DEPTH:
                    pending.pop(0)()
            for op in pending:
                op()

            # final: z over p_31 for both halves
            for h in range(2):
                zf = wp.tile([128, H], f32, tag="w")
                mmz(zf, zsm, p_prev[h], H, start=True, stop=True)
                rzfr = rzp.tile([128, H], bf16, tag="rzr")
                with nc.allow_low_precision(
                        reason="rz in bf16; ln(rz) is self-consistent"):
                    nc.vector.reciprocal(rzfr, zf)
                rzf = rzp.tile([128, H], bf16, tag="rzc")
                nc.vector.tensor_scalar(rzf, rzfr, float(2.0 ** 40), None,
                                        ALU.min)
                lnz = lnp.tile([128, H], bf16, tag="ln")
                nc.scalar.activation(
                    lnz, rzf, AF.Ln, accum_out=logacc[:, 16 + h:17 + h])

            # gold transition total: <T_blkdiag, C> per partition
            trscr = sg.tile([128, 128], f32)
            nc.vector.tensor_tensor(trscr, cs, tbd, ALU.mult)
            nc.vector.tensor_reduce(
                combo[:, 2:3], trscr, axis=mybir.AxisListType.X, op=ALU.add)

            nc.vector.tensor_reduce(
                combo[:, 0:1], logacc, axis=mybir.AxisListType.X, op=ALU.add)
            nc.vector.tensor_reduce(
                combo[:, 1:2], emitacc, axis=mybir.AxisListType.X, op=ALU.add)
            resw = wp.tile([128, 4], f32, tag="w")
            res = resw[0:4, 0:1]
            nc.tensor.matmul(res, lhsT=combo, rhs=ones,
                             start=True, stop=True)
            outsb = sg.tile([4, 1], f32)
            nc.vector.tensor_copy(out=outsb, in_=res)
            nc.sync.dma_start(out=OUTd.ap(), in_=outsb)

    nc.compile()
    return nc


def _get_program(batch=B):
    key = ("nc", batch)
    if key not in _cache:
        _cache[key] = _build_program(batch)
    return _cache[key]


def _make_in_maps(X, y, W, T, batch=B):
    global SCORE_DR
    import ml_dtypes
    fp8 = ml_dtypes.float8_e4m3
    bf = ml_dtypes.bfloat16
    GB = batch // GROUPS
    H = GB // 2

    X = np.asarray(X, dtype=np.float32)[:batch]
    y = np.asarray(y)[:batch]

    Xq = X.astype(fp8)
    if SCORE_DR:
        # fp8, DoubleRow layout [p=64, (t, h, i, g, c)], feature f = p+64i
        Xv = Xq.reshape(GROUPS, 2, H, L, 2, 64)        # g h c t i p
        XD = np.ascontiguousarray(
            Xv.transpose(5, 3, 1, 4, 0, 2)).reshape(64, -1)
    else:
        # fp8, plain layout [f=128, (t, h, g, c)]
        Xv = Xq.reshape(GROUPS, 2, H, L, 128)          # g h c t f
        XD = np.ascontiguousarray(
            Xv.transpose(4, 3, 1, 0, 2)).reshape(128, -1)

    # masks: one-hot of y in bf16, [p=(g,k32), (t, h, c)]
    yv = y.reshape(GROUPS, 2, H, L)                    # g h c t
    oh = (yv[..., None] == np.arange(32)).astype(bf)   # g h c t k
    MD = np.ascontiguousarray(oh.transpose(0, 4, 3, 1, 2)).reshape(128, -1)

    Wf = np.asarray(W, dtype=np.float32)
    if SCORE_DR:
        # fp8, per-group DoubleRow weights [p=64, (g, i, col128)],
        # zero outside group g's 32-partition block
        Wq = np.zeros((64, GROUPS, 2, 128), dtype=fp8)
        for g in range(GROUPS):
            Wq[:, g, 0, 32 * g:32 * g + K] = Wf[:64].astype(fp8)
            Wq[:, g, 1, 32 * g:32 * g + K] = Wf[64:].astype(fp8)
        WD = Wq.reshape(64, GROUPS * 256)
    else:
        Wq = np.zeros((128, 32), dtype=bf)
        Wq[:, :K] = Wf.astype(bf)
        WD = Wq

    # gold transition pair counts, block-diag per group [128, 128] f32
    CS = np.zeros((GROUPS, 32, 32), dtype=np.float32)
    yg = y.reshape(GROUPS, GB, L).astype(np.int64)
    for g in range(GROUPS):
        np.add.at(CS[g], (yg[g][:, :-1].ravel(), yg[g][:, 1:].ravel()), 1.0)
    CSf = np.zeros((128, 128), dtype=np.float32)
    for g in range(GROUPS):
        CSf[32 * g:32 * g + 32, 32 * g:32 * g + 32] = CS[g]

    TB = np.full((128, 128), -100.0, dtype=bf)
    Tb = np.asarray(T, dtype=np.float32).astype(bf)
    for g in range(GROUPS):
        TB[32 * g:32 * g + K, 32 * g:32 * g + K] = Tb
    return [{
        "XD": XD,
        "MD": MD,
        "WD": np.ascontiguousarray(WD),
        "TB": TB,
        "CS": CSf,
    }]


def _combine(results, batch=B):
    o = np.asarray(results[0]["out"], dtype=np.float64)
    # o = [32 * sum ln rz, emit, trans, 0]
    total = o[1, 0] + o[2, 0] + o[0, 0] / 32.0 - batch * L * SHIFT
    return np.float32(total / batch)


def kernel(X, y, W, T):
    from concourse.bass_utils import run_bass_kernel_spmd
    nc = _get_program()
    in_maps = _make_in_maps(X, y, W, T)
    res = run_bass_kernel_spmd(nc, in_maps, list(range(N_CORES)))
    return _combine(res.results)
